# revision 4
# baseline (speedup 1.0000x reference)
"""Trainium2 Bass kernel: 2-layer heterogeneous GCN + document aggregation,
8-core SPMD.

v2 design (vs v1's AllGather-everything):
  - Phase A (identity GCN dense layers) is REPLICATED: every core computes the
    full l11/l12 node tables from full (host-rotated) x inputs. Redundant
    compute + HBM reads are far cheaper than the 275-470us AllGathers they
    replace.
  - Each core's x tables are ROTATED so its own node shard sits at rows
    [0:S*], keeping all addressing core-independent (SPMD single program).
  - Phase B (e11/e22 spmm + 2nd GCN layer) is dst-sharded as in v1, but
    gathers from the LOCAL full tables - zero communication.
  - Phase C (e01/e02 doc aggregation) is re-sharded BY EDGE SOURCE: gathers
    hit the local l21c/l2all shard tables; each core accumulates partials for
    ALL doc nodes, then one ReduceScatter(+) per accumulator delivers summed
    per-core doc shards. 2x ~50us RS replaces 2x ~450us AllGathers.
  - All node tables, gather traffic, and S one-hot matmuls in bf16
    (4x PE throughput, ~half the DMA bytes); accumulation stays f32 in PSUM;
    final norm in f32. rel_err budget is 2e-2; bf16 lands ~1e-3.

Self-contained: hardcodes all shapes. kernel(**inputs) -> (doc, doc_svd).
"""

from contextlib import ExitStack

import numpy as np

import concourse.bacc as bacc
import concourse.mybir as mybir
from concourse.tile import TileContext
from concourse.bass_utils import run_bass_kernel_spmd

F32 = mybir.dt.float32
BF16 = mybir.dt.bfloat16
I16 = mybir.dt.int16
AF = mybir.ActivationFunctionType
ALU = mybir.AluOpType
NP_BF16 = mybir.dt.np(mybir.dt.bfloat16)

P = 128
NC = 8
SINGLE_PACKET = True
GP_BUFS = 3   # in-flight gather batches (4+ risks SWDGE ring overflow crash)
SP_BUFS = 16  # one-hot S tiles in flight
DMA_SCRATCH = 16384
IDX_BATCH = 1024  # gather rows per dma_gather call (2048 crashes ucode)
PS_BUFS = 2
EPS = 1e-9
KNOCKOUT = set()  # timing experiments: subsets of {"gather", "mm", "dve", "rs"}

# problem sizes
N0, N1, N2 = 10000, 50000, 30000
D, O, DW = 256, 128, 128

S0, S0P = N0 // NC, 1280   # doc shard: 1250 real rows, 10 tiles
S1, S1P = N1 // NC, 6272   # type-1 node shard: 6250 real, 49 tiles
S2, S2P = N2 // NC, 3840   # type-2 node shard: 3750 real, 30 tiles
T0, T1, T2 = S0P // P, S1P // P, S2P // P
N1F, N2F = 50176, 30720    # full padded tables; 392 / 240 tiles
TF1, TF2 = N1F // P, N2F // P
HALF1 = N1F // 2           # 25088: int16-safe half boundary of l11 table
ND, TD = NC * S0P, NC * S0P // P  # global padded doc space: 10240 rows, 80 tiles
TB1, TB2 = 28, 24          # phase-A streaming block sizes (tiles)


# ---------------------------------------------------------------------------
# Host-side edge preprocessing
# ---------------------------------------------------------------------------

class EdgePlan:
    """Static (core-independent) schedule + per-core data arrays for one edge
    set.

    core[e]     - core that processes edge e
    table_row[e]- row to gather in that core's (local) source table
    dst_row[e]  - accumulator row on that core, in [0, n_tiles*128)
    """

    def __init__(self, core, table_row, dst_row, w, n_tiles, half_bound):
        core = np.asarray(core, np.int64)
        table_row = np.asarray(table_row, np.int64)
        dst_row = np.asarray(dst_row, np.int64)
        w = np.asarray(w, np.float32)
        E = len(core)
        tile_id = dst_row // P
        dst_rel = (dst_row % P).astype(np.float32)
        if half_bound:
            half = (table_row >= half_bound).astype(np.int64)
            idx_rel = table_row - half * half_bound
            n_halves = 2
        else:
            half = np.zeros(E, np.int64)
            idx_rel = table_row
            n_halves = 1
        assert idx_rel.max() < 32768
        key = (core * n_halves + half) * n_tiles + tile_id
        counts = np.bincount(key, minlength=NC * n_halves * n_tiles).reshape(
            NC, n_halves, n_tiles
        )
        n_chunks = (-(-counts // P)).max(axis=0)  # [n_halves, n_tiles] cross-core max
        flat = n_chunks.reshape(-1)
        coff = np.concatenate([[0], np.cumsum(flat)[:-1]]).reshape(n_halves, n_tiles)
        total_chunks = int(flat.sum())

        order = np.argsort(key, kind="stable")
        key_s = key[order]
        grp_first = np.searchsorted(key_s, np.arange(NC * n_halves * n_tiles), "left")
        pos_in_grp = np.arange(E) - grp_first[key_s]
        spos = coff[half[order], tile_id[order]] * P + pos_in_grp

        idx_flat = np.zeros((NC, total_chunks * P), np.int16)
        w_flat = np.zeros((NC, total_chunks * P), np.float32)
        rel_flat = np.zeros((NC, total_chunks * P), np.float32)
        c_s = core[order]
        idx_flat[c_s, spos] = idx_rel[order].astype(np.int16)
        w_flat[c_s, spos] = w[order]
        rel_flat[c_s, spos] = dst_rel[order]

        self.n_halves = n_halves
        self.n_tiles = n_tiles
        self.n_chunks = n_chunks
        self.total_chunks = total_chunks
        self.half_nchunks = [int(n_chunks[h].sum()) for h in range(n_halves)]
        self.half_chunk_base = np.concatenate([[0], np.cumsum(self.half_nchunks)])
        self.sched = []  # per half: list of (tile, first, last) per chunk
        for h in range(n_halves):
            s = []
            for t in range(n_tiles):
                nct = int(n_chunks[h][t])
                for k in range(nct):
                    s.append((t, k == 0, k == nct - 1))
            self.sched.append(s)
        self.idx_wrapped = []  # [core][half] -> [128, Lh//16] int16
        self.w_wrapped = np.zeros((NC, P, total_chunks), np.float32)
        self.rel_wrapped = np.zeros((NC, P, total_chunks), np.float32)
        for c in range(NC):
            per_half = []
            for h in range(n_halves):
                lo = int(self.half_chunk_base[h]) * P
                hi = int(self.half_chunk_base[h + 1]) * P
                seg = idx_flat[c, lo:hi]
                wr = (np.tile(seg.reshape(-1, 16).T, (8, 1)) if hi > lo
                      else np.zeros((P, 0), np.int16))
                per_half.append(np.ascontiguousarray(wr))
            self.idx_wrapped.append(per_half)
            self.w_wrapped[c] = w_flat[c].reshape(total_chunks, P).T
            self.rel_wrapped[c] = rel_flat[c].reshape(total_chunks, P).T


# ---------------------------------------------------------------------------
# Device program pieces
# ---------------------------------------------------------------------------

def _dense_stream(nc, tc, psp, xT_in, n_tiles, tb, wk0, wk1, b_row, ones,
                  out_dram, name):
    """out_dram[(t p), :] = relu(xT[:, t-tile].T @ W + b), streamed in blocks
    of tb tiles. All bf16; PSUM accumulation f32."""
    out_re = out_dram.rearrange("(t p) f -> p t f", p=P)
    nblk = n_tiles // tb
    assert nblk * tb == n_tiles
    with (
        tc.tile_pool(name=f"{name}x", bufs=3) as xp,
        tc.tile_pool(name=f"{name}o", bufs=3) as op,
    ):
        for b in range(nblk):
            xa0 = xp.tile([P, tb * P], BF16, tag="xa0")
            xa1 = xp.tile([P, tb * P], BF16, tag="xa1")
            sl = slice(b * tb * P, (b + 1) * tb * P)
            nc.sync.dma_start(xa0[:], xT_in[0:P, sl])
            nc.sync.dma_start(xa1[:], xT_in[P : 2 * P, sl])
            outb = op.tile([P, tb, O], BF16, tag="ob")
            for m in range(tb):
                ps = psp.tile([P, O], F32, tag="psA")
                msl = slice(m * P, (m + 1) * P)
                nc.tensor.matmul(ps[:], lhsT=xa0[:, msl], rhs=wk0[:],
                                 start=True, stop=False)
                nc.tensor.matmul(ps[:], lhsT=xa1[:, msl], rhs=wk1[:],
                                 start=False, stop=False)
                nc.tensor.matmul(ps[:], lhsT=ones[:1, :], rhs=b_row[:1, :],
                                 start=False, stop=True)
                nc.scalar.activation(outb[:, m, :], ps[:], AF.Relu)
            nc.sync.dma_start(out_re[:, b * tb : (b + 1) * tb, :], outb[:])


def _spmm(nc, gpool, spool, psp, plan, tables, idx_tiles, w_col, rel_col, iota,
          accN, accT, width, gtag, pstag):
    """Shared spmm walker. Per 128-edge chunk builds S = onehot(dst_rel)*w
    (bf16), then:
      accT given (Option T, width==P): psum[feat, dst] += G_chunk.T @ S
      accN given (Option N): psum[dst, 0:width] += S.T @ G_chunk -> accN[:, t, :]
    Halves accumulate into f32 SBUF acc via copy-then-add."""
    n_tiles = plan.n_tiles
    written = [False] * n_tiles
    gchunk = 0
    for h in range(plan.n_halves):
        nch_h = plan.half_nchunks[h]
        if nch_h == 0:
            continue
        Lh = nch_h * P
        sched = plan.sched[h]
        table_ap, step = tables[h]
        ci = 0
        psum = None
        for b0 in range(0, Lh, IDX_BATCH):
            nidx = min(IDX_BATCH, Lh - b0)
            nch = nidx // P
            gt = gpool.tile([P, IDX_BATCH // P, width], BF16, tag=gtag)
            if "gather" not in KNOCKOUT:
                nc.gpsimd.dma_gather(
                    gt[:, :nch, :], table_ap,
                    idx_tiles[h][:, b0 // 16 : (b0 + nidx) // 16],
                    nidx, nidx, width, elem_step=step, single_packet=SINGLE_PACKET,
                )
            else:
                nc.vector.memset(gt[:, :1, :8], 0.0)
            for k in range(nch):
                t, first, last = sched[ci]
                if first:
                    psum = psp.tile([P, width], F32, tag=pstag)
                S = spool.tile([P, P], BF16, tag="S")
                if "dve" not in KNOCKOUT:
                    nc.vector.tensor_scalar(
                        S[:], iota[:], rel_col[:, gchunk : gchunk + 1],
                        w_col[:, gchunk : gchunk + 1], ALU.is_equal, ALU.mult,
                    )
                else:
                    nc.vector.memset(S[:, :8], 0.0)
                if "mm" not in KNOCKOUT:
                    if accN is not None:
                        nc.tensor.matmul(psum[:], lhsT=S[:], rhs=gt[:, k, :],
                                         start=first, stop=last)
                    else:
                        nc.tensor.matmul(psum[:], lhsT=gt[:, k, :], rhs=S[:],
                                         start=first, stop=last)
                elif first:
                    nc.tensor.matmul(psum[:, 0:P], lhsT=iota[:1, :], rhs=iota[:1, :],
                                     start=True, stop=True)
                if last:
                    sl = (accN[:, t, :] if accN is not None
                          else accT[:, t * P : (t + 1) * P])
                    if not written[t]:
                        nc.scalar.activation(sl, psum[:], AF.Copy)
                        written[t] = True
                    else:
                        nc.vector.tensor_tensor(sl, psum[:], sl, ALU.add)
                ci += 1
                gchunk += 1
    for t in range(n_tiles):
        if not written[t]:
            sl = accN[:, t, :] if accN is not None else accT[:, t * P : (t + 1) * P]
            nc.vector.memset(sl, 0.0)


def _gcn_second(nc, psp, accT, w_t, b_row, ones, outc, n_tiles):
    """outc[:, t, 0:O] = relu(accT_t.T @ W + b). accT/W f32, outc bf16."""
    for t in range(n_tiles):
        ps = psp.tile([P, P], F32, tag="ps2")
        nc.tensor.matmul(ps[:], lhsT=accT[:, t * P : (t + 1) * P], rhs=w_t[:],
                         start=True, stop=False)
        nc.tensor.matmul(ps[:], lhsT=ones[:1, :], rhs=b_row[:1, :], start=False,
                         stop=True)
        nc.scalar.activation(outc[:, t, 0:O], ps[:], AF.Relu)


def _edge_phase_loads(nc, sb, plan, idx_in, w_in, rel_in, tagp):
    idx_t = []
    for h in range(plan.n_halves):
        it = sb.tile(list(plan.idx_wrapped[0][h].shape), I16, tag=f"{tagp}i{h}")
        nc.sync.dma_start(it[:], idx_in[h][:])
        idx_t.append(it)
    wct = sb.tile([P, plan.total_chunks], F32, tag=f"{tagp}w")
    relt = sb.tile([P, plan.total_chunks], F32, tag=f"{tagp}r")
    nc.sync.dma_start(wct[:], w_in[:])
    nc.sync.dma_start(relt[:], rel_in[:])
    return idx_t, wct, relt


# ---------------------------------------------------------------------------
# Full program
# ---------------------------------------------------------------------------

def build_program(p11, p22, p01, p02, repeat=1):
    nc = bacc.Bacc("TRN2", num_devices=NC, dynamic_dma_scratch_size=DMA_SCRATCH)

    x1T = nc.dram_tensor("x1T", [D, N1F], BF16, kind="ExternalInput")
    x2T = nc.dram_tensor("x2T", [D, N2F], BF16, kind="ExternalInput")
    wemb_sh = nc.dram_tensor("wemb_sh", [S2P, DW], BF16, kind="ExternalInput")
    w1a = nc.dram_tensor("w1a", [D, O], BF16, kind="ExternalInput")
    w2a = nc.dram_tensor("w2a", [D, O], BF16, kind="ExternalInput")
    w1b = nc.dram_tensor("w1b", [O, O], F32, kind="ExternalInput")
    w2b = nc.dram_tensor("w2b", [O, O], F32, kind="ExternalInput")
    biases_bf = nc.dram_tensor("biases_bf", [2, O], BF16, kind="ExternalInput")
    biases_f = nc.dram_tensor("biases_f", [2, O], F32, kind="ExternalInput")
    iota_in = nc.dram_tensor("iota", [P, P], BF16, kind="ExternalInput")
    onesb_in = nc.dram_tensor("onesb", [1, P], BF16, kind="ExternalInput")
    onesf_in = nc.dram_tensor("onesf", [1, P], F32, kind="ExternalInput")

    def edge_inputs(name, plan):
        idx = [
            nc.dram_tensor(f"{name}_idx{h}", list(plan.idx_wrapped[0][h].shape), I16,
                           kind="ExternalInput")
            for h in range(plan.n_halves)
        ]
        wv = nc.dram_tensor(f"{name}_w", [P, plan.total_chunks], F32,
                            kind="ExternalInput")
        rel = nc.dram_tensor(f"{name}_rel", [P, plan.total_chunks], F32,
                             kind="ExternalInput")
        return idx, wv, rel

    e11_in = edge_inputs("e11", p11)
    e22_in = edge_inputs("e22", p22)
    e01_in = edge_inputs("e01", p01)
    e02_in = edge_inputs("e02", p02)

    l11_full = nc.dram_tensor("l11_full", [N1F, O], BF16)
    l12_full = nc.dram_tensor("l12_full", [N2F, O], BF16)
    l21c_loc = nc.dram_tensor("l21c_loc", [S1P, 2 * O], BF16)
    l2all_loc = nc.dram_tensor("l2all_loc", [S2P, 3 * O], BF16)
    r0part = nc.dram_tensor("r0part", [ND, 2 * O], F32)
    r1part = nc.dram_tensor("r1part", [ND, 3 * O], F32)
    r0sum = nc.dram_tensor("r0sum", [S0P, 2 * O], F32)
    r1sum = nc.dram_tensor("r1sum", [S0P, 3 * O], F32)

    doc_loc = nc.dram_tensor("doc_local", [S0P, 2 * O + DW], F32,
                             kind="ExternalOutput")
    docsvd_loc = nc.dram_tensor("docsvd_local", [S0P, 2 * O + DW], F32,
                                kind="ExternalOutput")

    rg = [list(range(NC))]

    def rs(inp, outp):
        if "rs" not in KNOCKOUT:
            nc.gpsimd.collective_compute(
                "ReduceScatter", ALU.add, replica_groups=rg,
                ins=[inp[:]], outs=[outp[:]])

    def rearr(dram_ap):
        return dram_ap.rearrange("(t p) f -> p t f", p=P)

    with TileContext(nc) as tc:
        with tc.tile_pool(name="const", bufs=1) as cp:
            iota = cp.tile([P, P], BF16)
            onesb = cp.tile([1, P], BF16)
            onesf = cp.tile([1, P], F32)
            nc.sync.dma_start(iota[:], iota_in[:])
            nc.sync.dma_start(onesb[:], onesb_in[:])
            nc.sync.dma_start(onesf[:], onesf_in[:])
            w1a0 = cp.tile([P, O], BF16); nc.sync.dma_start(w1a0[:], w1a[0:P, :])
            w1a1 = cp.tile([P, O], BF16); nc.sync.dma_start(w1a1[:], w1a[P:D, :])
            w2a0 = cp.tile([P, O], BF16); nc.sync.dma_start(w2a0[:], w2a[0:P, :])
            w2a1 = cp.tile([P, O], BF16); nc.sync.dma_start(w2a1[:], w2a[P:D, :])
            w1bt = cp.tile([O, O], F32); nc.sync.dma_start(w1bt[:], w1b[:])
            w2bt = cp.tile([O, O], F32); nc.sync.dma_start(w2bt[:], w2b[:])
            b1a_t = cp.tile([1, O], BF16)
            nc.sync.dma_start(b1a_t[:], biases_bf[0:1, :])
            b2a_t = cp.tile([1, O], BF16)
            nc.sync.dma_start(b2a_t[:], biases_bf[1:2, :])
            b1b_t = cp.tile([1, O], F32)
            nc.sync.dma_start(b1b_t[:], biases_f[0:1, :])
            b2b_t = cp.tile([1, O], F32)
            nc.sync.dma_start(b2b_t[:], biases_f[1:2, :])

            def emit_body():
                with ExitStack() as ab:
                    psp = ab.enter_context(
                        tc.tile_pool(name="psum", bufs=PS_BUFS, space="PSUM"))
                    pspw = ab.enter_context(
                        tc.tile_pool(name="psumW", bufs=2, space="PSUM"))

                    # ---- phase A: replicated dense identity-GCN layers ----
                    _dense_stream(nc, tc, psp, x1T, TF1, TB1, w1a0, w1a1,
                                  b1a_t[:], onesb, l11_full, "A1")
                    _dense_stream(nc, tc, psp, x2T, TF2, TB2, w2a0, w2a1,
                                  b2a_t[:], onesb, l12_full, "A2")

                    l11_halves = [(l11_full[0:HALF1, :], None),
                                  (l11_full[HALF1 : 2 * HALF1, :], None)]

                    # ---- phase B1: spmm(e11) -> @W1b -> l21c = [l2_1 | l1_1] ----
                    with (
                        tc.tile_pool(name="phB1", bufs=1) as sb1,
                        tc.tile_pool(name="gpB1", bufs=GP_BUFS) as gp1,
                        tc.tile_pool(name="spB1", bufs=SP_BUFS) as sp1,
                    ):
                        idx_t, wct, relt = _edge_phase_loads(
                            nc, sb1, p11, e11_in[0], e11_in[1], e11_in[2], tagp="a")
                        accT = sb1.tile([P, S1P], F32, tag="accT")
                        _spmm(nc, gp1, sp1, psp, p11, l11_halves, idx_t, wct, relt,
                              iota, None, accT, P, "gT", "psT")
                        outc = sb1.tile([P, T1, O], BF16, tag="outc")
                        _gcn_second(nc, pspw, accT, w1bt, b1b_t[:], onesf, outc, T1)
                        nc.sync.dma_start(rearr(l21c_loc[:])[:, :, 0:O], outc[:])
                        # own shard's l1_1 lives at l11_full rows [0:S1P] (rotation)
                        nc.sync.dma_start(l21c_loc[:, O : 2 * O], l11_full[0:S1P, :])

                    # ---- phase B2: spmm(e22) -> @W2b -> l2all = [l2_2|l1_2|wemb] --
                    with (
                        tc.tile_pool(name="phB2", bufs=1) as sb2,
                        tc.tile_pool(name="gpB2", bufs=GP_BUFS) as gp2,
                        tc.tile_pool(name="spB2", bufs=SP_BUFS) as sp2,
                    ):
                        idx_t, wct, relt = _edge_phase_loads(
                            nc, sb2, p22, e22_in[0], e22_in[1], e22_in[2], tagp="b")
                        accT = sb2.tile([P, S2P], F32, tag="accT")
                        _spmm(nc, gp2, sp2, psp, p22, [(l12_full[:], None)],
                              idx_t, wct, relt, iota, None, accT, P, "gT", "psT")
                        outc2 = sb2.tile([P, T2, O], BF16, tag="outc2")
                        _gcn_second(nc, pspw, accT, w2bt, b2b_t[:], onesf, outc2, T2)
                        nc.sync.dma_start(rearr(l2all_loc[:])[:, :, 0:O], outc2[:])
                        nc.sync.dma_start(l2all_loc[:, O : 2 * O], l12_full[0:S2P, :])
                        nc.sync.dma_start(l2all_loc[:, 2 * O : 3 * O], wemb_sh[:])

                    # ---- phase C1: src-sharded spmm(e01) over all doc nodes ----
                    with (
                        tc.tile_pool(name="phC1", bufs=1) as sc1,
                        tc.tile_pool(name="psC1", bufs=PS_BUFS, space="PSUM") as psc1,
                        tc.tile_pool(name="gpC1", bufs=GP_BUFS) as gpc1,
                        tc.tile_pool(name="spC1", bufs=SP_BUFS) as spc1,
                    ):
                        idx_t, wct, relt = _edge_phase_loads(
                            nc, sc1, p01, e01_in[0], e01_in[1], e01_in[2], tagp="c")
                        acc01 = sc1.tile([P, TD, 2 * O], F32, tag="acc01")
                        _spmm(nc, gpc1, spc1, psc1, p01, [(l21c_loc[:], None)],
                              idx_t, wct, relt, iota, acc01[:], None, 2 * O,
                              "g01", "ps01")
                        nc.sync.dma_start(rearr(r0part[:]), acc01[:])
                    rs(r0part, r0sum)

                    # ---- phase C2: src-sharded spmm(e02) ----
                    with (
                        tc.tile_pool(name="phC2", bufs=1) as sc2,
                        tc.tile_pool(name="psC2", bufs=PS_BUFS, space="PSUM") as psc2,
                        tc.tile_pool(name="gpC2", bufs=GP_BUFS) as gpc2,
                        tc.tile_pool(name="spC2", bufs=SP_BUFS) as spc2,
                    ):
                        idx_t, wct, relt = _edge_phase_loads(
                            nc, sc2, p02, e02_in[0], e02_in[1], e02_in[2], tagp="d")
                        acc02 = sc2.tile([P, TD, 3 * O], F32, tag="acc02")
                        _spmm(nc, gpc2, spc2, psc2, p02, [(l2all_loc[:], None)],
                              idx_t, wct, relt, iota, acc02[:], None, 3 * O,
                              "g02", "ps02")
                        nc.sync.dma_start(rearr(r1part[:]), acc02[:])
                    rs(r1part, r1sum)

                # ---- phase D: l2norm + output ----
                with tc.tile_pool(name="phD", bufs=1) as sd:
                    a01 = sd.tile([P, T0, 2 * O], F32, tag="a01")
                    a02 = sd.tile([P, T0, 3 * O], F32, tag="a02")
                    nc.sync.dma_start(a01[:], rearr(r0sum[:]))
                    nc.sync.dma_start(a02[:], rearr(r1sum[:]))
                    docb = sd.tile([P, T0, 2 * O + DW], F32, tag="docb")
                    docsb = sd.tile([P, T0, 2 * O + DW], F32, tag="docsb")

                    def norm_scale(acc_slices, out_writes):
                        ss_total = None
                        for i, (s_ap, wdt) in enumerate(acc_slices):
                            sq = sd.tile([P, wdt], F32, tag=f"sq{i}")
                            ss = sd.tile([P, 1], F32, tag=f"ss{i}")
                            nc.scalar.activation(sq[:], s_ap, AF.Square,
                                                 accum_out=ss[:])
                            if ss_total is None:
                                ss_total = ss
                            else:
                                nc.vector.tensor_tensor(
                                    ss_total[:], ss[:], ss_total[:], ALU.add)
                        nrm = sd.tile([P, 1], F32, tag="nrm")
                        nc.scalar.activation(nrm[:], ss_total[:], AF.Sqrt)
                        nc.vector.tensor_scalar_add(nrm[:], nrm[:], EPS)
                        rn = sd.tile([P, 1], F32, tag="rn")
                        nc.vector.reciprocal(rn[:], nrm[:])
                        for dst_ap, s_ap in out_writes:
                            nc.vector.tensor_scalar_mul(dst_ap, s_ap, rn[:])

                    for t in range(T0):
                        c01 = a01[:, t, :]
                        c02 = a02[:, t, :]
                        # doc = [norm(r0) | norm([l22|wemb])]
                        norm_scale([(c01[:, 0:O], O)],
                                   [(docb[:, t, 0:O], c01[:, 0:O])])
                        norm_scale(
                            [(c02[:, 0:O], O), (c02[:, 2 * O : 3 * O], O)],
                            [(docb[:, t, O : 2 * O], c02[:, 0:O]),
                             (docb[:, t, 2 * O : 3 * O], c02[:, 2 * O : 3 * O])])
                        # doc_svd = [norm(r0s) | norm([l12|wemb])]
                        norm_scale([(c01[:, O : 2 * O], O)],
                                   [(docsb[:, t, 0:O], c01[:, O : 2 * O])])
                        norm_scale([(c02[:, O : 3 * O], 2 * O)],
                                   [(docsb[:, t, O : 3 * O], c02[:, O : 3 * O])])
                    nc.sync.dma_start(rearr(doc_loc[:]), docb[:])
                    nc.sync.dma_start(rearr(docsvd_loc[:]), docsb[:])

            for _ in range(repeat):
                emit_body()

    nc.compile()
    return nc


# ---------------------------------------------------------------------------
# Host wrapper
# ---------------------------------------------------------------------------

_CACHE = {}


def _make_plans(inputs):
    e11_src = np.asarray(inputs["e11_src"], np.int64)
    e11_dst = np.asarray(inputs["e11_dst"], np.int64)
    e22_src = np.asarray(inputs["e22_src"], np.int64)
    e22_dst = np.asarray(inputs["e22_dst"], np.int64)
    e01_src = np.asarray(inputs["e01_src"], np.int64)
    e01_dst = np.asarray(inputs["e01_dst"], np.int64)
    e02_src = np.asarray(inputs["e02_src"], np.int64)
    e02_dst = np.asarray(inputs["e02_dst"], np.int64)

    c11 = e11_dst // S1
    p11 = EdgePlan(c11, (e11_src - c11 * S1) % N1, e11_dst - c11 * S1,
                   inputs["e11_w"], T1, HALF1)
    c22 = e22_dst // S2
    p22 = EdgePlan(c22, (e22_src - c22 * S2) % N2, e22_dst - c22 * S2,
                   inputs["e22_w"], T2, None)
    c01 = e01_src // S1
    p01 = EdgePlan(c01, e01_src % S1,
                   (e01_dst // S0) * S0P + e01_dst % S0, inputs["e01_w"], TD, None)
    c02 = e02_src // S2
    p02 = EdgePlan(c02, e02_src % S2,
                   (e02_dst // S0) * S0P + e02_dst % S0, inputs["e02_w"], TD, None)
    return p11, p22, p01, p02


def _prep(inputs):
    x1 = np.asarray(inputs["x1"], np.float32)
    x2 = np.asarray(inputs["x2"], np.float32)
    wemb = np.asarray(inputs["word_emb"], np.float32)

    p11, p22, p01, p02 = _make_plans(inputs)

    iota = np.tile(np.arange(P, dtype=np.float32), (P, 1)).astype(NP_BF16)
    onesb = np.ones((1, P), NP_BF16)
    onesf = np.ones((1, P), np.float32)
    biases_bf = np.stack([
        np.asarray(inputs["b1a"], np.float32), np.asarray(inputs["b2a"], np.float32),
    ]).astype(NP_BF16)
    biases_f = np.stack([
        np.asarray(inputs["b1b"], np.float32), np.asarray(inputs["b2b"], np.float32),
    ])

    in_maps = []
    for c in range(NC):
        # rotate so core c's shard sits at rows [0:S*]
        x1r = np.concatenate([x1[c * S1 :], x1[: c * S1]], axis=0)
        x2r = np.concatenate([x2[c * S2 :], x2[: c * S2]], axis=0)
        x1T = np.zeros((D, N1F), NP_BF16)
        x1T[:, :N1] = x1r.T.astype(NP_BF16)
        x2T = np.zeros((D, N2F), NP_BF16)
        x2T[:, :N2] = x2r.T.astype(NP_BF16)
        wsh = np.zeros((S2P, DW), NP_BF16)
        wsh[:S2] = wemb[c * S2 : (c + 1) * S2].astype(NP_BF16)
        m = {
            "x1T": x1T, "x2T": x2T, "wemb_sh": wsh,
            "w1a": np.asarray(inputs["W1a"], np.float32).astype(NP_BF16),
            "w2a": np.asarray(inputs["W2a"], np.float32).astype(NP_BF16),
            "w1b": np.asarray(inputs["W1b"], np.float32),
            "w2b": np.asarray(inputs["W2b"], np.float32),
            "biases_bf": biases_bf, "biases_f": biases_f,
            "iota": iota, "onesb": onesb, "onesf": onesf,
        }
        for name, plan in (("e11", p11), ("e22", p22), ("e01", p01), ("e02", p02)):
            for h in range(plan.n_halves):
                m[f"{name}_idx{h}"] = plan.idx_wrapped[c][h]
            m[f"{name}_w"] = np.ascontiguousarray(plan.w_wrapped[c])
            m[f"{name}_rel"] = np.ascontiguousarray(plan.rel_wrapped[c])
        in_maps.append(m)
    return (p11, p22, p01, p02), in_maps


def get_compiled(inputs):
    plans, in_maps = _prep(inputs)
    key = tuple(p.total_chunks for p in plans) + tuple(
        tuple(p.n_chunks.reshape(-1).tolist()) for p in plans
    )
    if key not in _CACHE:
        _CACHE[key] = build_program(*plans)
    return _CACHE[key], in_maps


def kernel(**inputs):
    nc, in_maps = get_compiled(inputs)
    res = run_bass_kernel_spmd(nc, in_maps, core_ids=list(range(NC)), trace=False)
    doc = np.concatenate([res.results[c]["doc_local"][:S0] for c in range(NC)], axis=0)
    dsvd = np.concatenate([res.results[c]["docsvd_local"][:S0] for c in range(NC)],
                          axis=0)
    return (doc[:N0], dsvd[:N0])
